# revision 14
# baseline (speedup 1.0000x reference)
"""CiaoSR query-decode kernel for 8 Trainium2 NeuronCores.

Strategy (data-parallel over query points Q):
  - gen_feat (convs + imnet MLP over the 128x128 feature map) is sharded by
    image rows (16 rows/core); the per-pixel feature map [16384, 64] is
    AllGathered so every core holds the full map.
  - The two grid samples per query share iy (vx=-1 only), so each query
    needs rows idx0 = iy*W+ix0 and idx1 = iy*W+ix1 of the pixel-major map.
    Rows are fetched with one [128-partition, 1-offset] indirect DMA per
    128-query block per sample offset.
  - kk/vv MLPs (68->256->576) run as PE matmuls; per-query area weights are
    folded into the hidden activations so the two sample offsets accumulate
    in one PSUM tile; kk_b2/vv_b2 are added on the host (their reference
    coefficient a0+a1 == 1 up to 1 ulp).
"""

import numpy as np

import bass_rust
import concourse.bass as bass
import concourse.mybir as mybir
import concourse.tile as tile
from concourse import bacc
from concourse.bass_utils import run_bass_kernel_spmd
from concourse.masks import make_identity
from concourse.vector_clock import ScopedClock

F32 = mybir.dt.float32
I32 = mybir.dt.int32
Op = mybir.AluOpType
Act = mybir.ActivationFunctionType

H = W = 128
C = 64
Q = 65536
NCORES = 8
QS = Q // NCORES          # 8192 queries per core
NB = QS // 128            # 64 blocks of 128 queries
ROWS = H // NCORES        # 16 image rows per core
PIX = ROWS * W            # 2048 pixels per core
RX = 1.0 / H
EPS = 1e-6
PI = float(np.pi)

# ---------------------------------------------------------------------------
# walrus wait-capacity workaround: hoist excess per-instruction sem waits
# onto standalone event-semaphore nops (this walrus accepts 1 wait per
# instruction, 2 on EventSemaphore).
_wait_ctr = [0]


def _legalize_waits(nc):
    for fn in nc.m.functions:
        for bb in fn.blocks:
            insts = bb.instructions
            out = []
            changed = False
            for inst in insts:
                si = inst.sync_info
                waits = list(si.on_wait) if si is not None else []
                cap = 2 if isinstance(inst, mybir.InstEventSemaphore) else 1
                if len(waits) > cap:
                    changed = True
                    extra, keep = waits[:-cap], waits[-cap:]
                    for i in range(0, len(extra), 2):
                        _wait_ctr[0] += 1
                        out.append(
                            mybir.InstEventSemaphore(
                                name=f"I-waitfix-{_wait_ctr[0]}",
                                engine=inst.engine,
                                ins=[],
                                outs=[],
                                sync_info=bass_rust.SyncInfo(
                                    on_wait=extra[i : i + 2], on_update=[]
                                ),
                            )
                        )
                    si.on_wait = keep
                    inst.sync_info = si
                out.append(inst)
            if changed:
                bb.instructions = out


_patched = [False]


def _patch_tile():
    if _patched[0]:
        return
    _patched[0] = True
    orig_exit = tile.TileContext.__exit__

    def patched_exit(self, exc_type, exc, tb):
        r = orig_exit(self, exc_type, exc, tb)
        if exc_type is None:
            _legalize_waits(self.nc)
        return r

    tile.TileContext.__exit__ = patched_exit


# ---------------------------------------------------------------------------
# program construction


def _build_program():
    _patch_tile()
    nc = bacc.Bacc("TRN2", target_bir_lowering=False, debug=False,
                   num_devices=NCORES)

    def ein(name, shape):
        return nc.dram_tensor(name, shape, F32, kind="ExternalInput")

    slab2 = ein("slab2", [128, ROWS, W + 2])
    slab_c = ein("slab_c", [64, ROWS, W + 2])
    wa1 = ein("wa1", [128, 3, 64])
    wb1 = ein("wb1", [64, 3, 64])
    wa2 = ein("wa2", [128, 3, 64])
    wb2 = ein("wb2", [64, 3, 64])
    cbias = ein("cbias", [64, 1])
    qbias = ein("qbias", [64, 1])
    w1i = ein("w1i", [64, 256])
    b1i = ein("b1i", [128, 2])
    w2i = ein("w2i", [128, 2, 64])
    b2i = ein("b2i", [1, 64])
    w1kk = ein("w1kk", [70, 256])
    w1vv = ein("w1vv", [70, 256])
    w2kk = ein("w2kk", [128, 2, 576])
    w2vv = ein("w2vv", [128, 2, 576])
    coordA = ein("coordA", [128, NB, 2])
    cellA = ein("cellA", [128, NB, 2])
    # selector: sel70.T @ xbuf broadcasts xbuf row 69 (the area weights)
    # to all 128 output partitions
    sel70_d = ein("sel70", [70, 128])
    out_d = nc.dram_tensor("out", [QS, 1152], F32, kind="ExternalOutput")

    with tile.TileContext(nc) as tc:
        with tc.tile_pool(name="const", bufs=1) as cp, \
             tc.tile_pool(name="dram", bufs=1, space="DRAM") as dp:
            featsh = dp.tile([PIX, C], F32)
            featfull = dp.tile([H * W, C], F32, addr_space="Shared")

            ident = cp.tile([128, 128], F32)
            make_identity(nc, ident[:])
            ones1 = cp.tile([1, 128], F32)
            nc.vector.memset(ones1[:], 1.0)


            def load(src, shape):
                t = cp.tile(shape, F32, name=src.name + "_sb")
                nc.sync.dma_start(t[:], src[(slice(None),) * len(shape)])
                return t

            slab2_s = load(slab2, [128, ROWS, W + 2])
            slabc_s = load(slab_c, [64, ROWS, W + 2])
            wa1_s = load(wa1, [128, 3, 64])
            wb1_s = load(wb1, [64, 3, 64])
            wa2_s = load(wa2, [128, 3, 64])
            wb2_s = load(wb2, [64, 3, 64])
            cbias_s = load(cbias, [64, 1])
            qbias_s = load(qbias, [64, 1])
            w1i_s = load(w1i, [64, 256])
            b1i_s = load(b1i, [128, 2])
            w2i_s = load(w2i, [128, 2, 64])
            b2i_s = load(b2i, [1, 64])
            w1kk_s = load(w1kk, [70, 256])
            w1vv_s = load(w1vv, [70, 256])
            w2kk_s = load(w2kk, [128, 2, 576])
            w2vv_s = load(w2vv, [128, 2, 576])
            coordA_s = load(coordA, [128, NB, 2])
            cellA_s = load(cellA, [128, NB, 2])
            sel70 = load(sel70_d, [70, 128])

            # ---------------- phase A: gen_feat ----------------
            with tc.tile_pool(name="ga", bufs=3) as ga, \
                 tc.tile_pool(name="gh", bufs=4) as gh, \
                 tc.tile_pool(name="pc", bufs=2, space="PSUM") as ppc, \
                 tc.tile_pool(name="pq", bufs=2, space="PSUM") as ppq, \
                 tc.tile_pool(name="phA", bufs=2, space="PSUM") as pph, \
                 tc.tile_pool(name="pfA", bufs=2, space="PSUM") as ppf:
                for yg in range(ROWS // 4):
                    y0 = yg * 4
                    pc = ppc.tile([64, 4, 128], F32, name="pc")
                    pq = ppq.tile([64, 4, 128], F32, name="pq")
                    for po, wa, wb in ((pc, wa1_s, wb1_s), (pq, wa2_s, wb2_s)):
                        for dx in range(3):
                            nc.tensor.matmul(
                                po[:], lhsT=wa[:, dx, :],
                                rhs=slab2_s[:, y0:y0 + 4, dx:dx + 128],
                                start=(dx == 0), stop=False)
                        for dx in range(3):
                            nc.tensor.matmul(
                                po[:], lhsT=wb[:, dx, :],
                                rhs=slabc_s[:, y0:y0 + 4, dx:dx + 128],
                                start=False, stop=(dx == 2))
                    # trig: rows 0-31 cos(pi*qf) (via sin(pi*(qf+0.5-2k))),
                    # rows 32-63 sin(pi*qf)
                    u = ga.tile([64, 4, 128], F32, name="u")
                    # u = pq + qbias
                    nc.vector.tensor_scalar(
                        out=u[:], in0=pq[:], scalar1=qbias_s[:, 0:1],
                        scalar2=None, op0=Op.add)
                    # cos half: add 0.5
                    nc.vector.tensor_scalar(
                        out=u[0:32], in0=u[0:32], scalar1=0.5, scalar2=None,
                        op0=Op.add)
                    uh = ga.tile([64, 4, 128], F32, name="uh")
                    nc.vector.tensor_scalar(
                        out=uh[:], in0=u[:], scalar1=0.5, scalar2=None,
                        op0=Op.mult)
                    ki = ga.tile([64, 4, 128], I32, name="ki")
                    nc.vector.tensor_copy(ki[:], uh[:])
                    kf = ga.tile([64, 4, 128], F32, name="kf")
                    nc.vector.tensor_copy(kf[:], ki[:])
                    r = ga.tile([64, 4, 128], F32, name="r")
                    nc.vector.scalar_tensor_tensor(
                        out=r[:], in0=kf[:], scalar=-2.0, in1=u[:],
                        op0=Op.mult, op1=Op.add)
                    cossin = ga.tile([64, 4, 128], F32, name="cossin")
                    nc.scalar.activation(cossin[:], r[:], Act.Sin, scale=PI)
                    x2 = ga.tile([64, 4, 128], F32, name="x2")
                    nc.vector.scalar_tensor_tensor(
                        out=x2[:], in0=pc[:], scalar=cbias_s[:, 0:1],
                        in1=cossin[:], op0=Op.add, op1=Op.mult)
                    hs = []
                    for m in range(2):
                        ph = pph.tile([128, 512], F32, name="phA")
                        nc.tensor.matmul(
                            ph[:], lhsT=w1i_s[:, m * 128:(m + 1) * 128],
                            rhs=x2[:].rearrange("c a b -> c (a b)"),
                            start=True, stop=True)
                        hm = gh.tile([128, 512], F32, name="hA")
                        nc.scalar.activation(hm[:], ph[:], Act.Relu,
                                             bias=b1i_s[:, m:m + 1])
                        hs.append(hm)
                    for sub in range(4):
                        pf = ppf.tile([128, 64], F32, name="pfA")
                        nc.tensor.matmul(
                            pf[:], lhsT=hs[0][:, sub * 128:(sub + 1) * 128],
                            rhs=w2i_s[:, 0, :], start=True, stop=False)
                        nc.tensor.matmul(
                            pf[:], lhsT=hs[1][:, sub * 128:(sub + 1) * 128],
                            rhs=w2i_s[:, 1, :], start=False, stop=False)
                        nc.tensor.matmul(
                            pf[:], lhsT=ones1[:], rhs=b2i_s[:],
                            start=False, stop=True)
                        ft = ga.tile([128, 64], F32, name="ft")
                        nc.scalar.copy(ft[:], pf[:])
                        r0 = yg * 512 + sub * 128
                        nc.sync.dma_start(featsh[r0:r0 + 128, :], ft[:])

            nc.gpsimd.collective_compute(
                "AllGather",
                Op.bypass,
                replica_groups=[list(range(NCORES))],
                ins=[featsh[:, :]],
                outs=[featfull[:, :]],
            )

            # ---------------- phase B: coord math ----------------
            with tc.tile_pool(name="cm", bufs=1) as cm, \
                 tc.tile_pool(name="cmt", bufs=2) as cmt:
                c0 = coordA_s[:, :, 0]
                c1 = coordA_s[:, :, 1]

                def new(name):
                    return cm.tile([128, NB], F32, name=name)

                def pixel_center_chain(csrc, off_const, name):
                    """returns (i_clipped_f32, qc) replicating jnp op order"""
                    t = new(name + "_t")
                    nc.vector.tensor_scalar(out=t[:], in0=csrc, scalar1=float(off_const),
                                            scalar2=None, op0=Op.add)
                    nc.vector.tensor_scalar(out=t[:], in0=t[:], scalar1=float(np.float32(EPS)),
                                            scalar2=None, op0=Op.add)
                    nc.vector.tensor_scalar(out=t[:], in0=t[:],
                                            scalar1=float(np.float32(-1 + EPS)),
                                            scalar2=float(np.float32(1 - EPS)),
                                            op0=Op.max, op1=Op.min)
                    nc.vector.tensor_scalar(out=t[:], in0=t[:], scalar1=1.0,
                                            scalar2=None, op0=Op.add)
                    nc.vector.tensor_scalar(out=t[:], in0=t[:], scalar1=128.0,
                                            scalar2=None, op0=Op.mult)
                    nc.vector.tensor_scalar(out=t[:], in0=t[:], scalar1=-1.0,
                                            scalar2=None, op0=Op.add)
                    nc.vector.tensor_scalar(out=t[:], in0=t[:], scalar1=0.5,
                                            scalar2=None, op0=Op.mult)
                    ti = cm.tile([128, NB], I32, name=name + "_i")
                    nc.vector.tensor_copy(ti[:], t[:])
                    tf = new(name + "_f")
                    nc.vector.tensor_copy(tf[:], ti[:])
                    nc.vector.tensor_scalar(out=tf[:], in0=tf[:], scalar1=0.0,
                                            scalar2=127.0, op0=Op.max, op1=Op.min)
                    qc = new(name + "_qc")
                    nc.vector.tensor_scalar(out=qc[:], in0=tf[:], scalar1=2.0,
                                            scalar2=1.0, op0=Op.mult, op1=Op.add)
                    nc.vector.tensor_scalar(out=qc[:], in0=qc[:],
                                            scalar1=float(np.float32(1.0 / 128.0)),
                                            scalar2=-1.0, op0=Op.mult, op1=Op.add)
                    return tf, qc

                iy_f, qcy = pixel_center_chain(c0, np.float32(-RX), "iy")
                ix0_f, qcx0 = pixel_center_chain(c1, np.float32(-RX), "ix0")
                ix1_f, qcx1 = pixel_center_chain(c1, np.float32(RX), "ix1")

                rel0 = new("rel0")
                nc.vector.tensor_tensor(out=rel0[:], in0=c0, in1=qcy[:],
                                        op=Op.subtract)
                nc.vector.tensor_scalar(out=rel0[:], in0=rel0[:], scalar1=128.0,
                                        scalar2=None, op0=Op.mult)
                rel1 = []
                for s, qcx in ((0, qcx0), (1, qcx1)):
                    rl = new(f"rel1_{s}")
                    nc.vector.tensor_tensor(out=rl[:], in0=c1, in1=qcx[:],
                                            op=Op.subtract)
                    nc.vector.tensor_scalar(out=rl[:], in0=rl[:], scalar1=128.0,
                                            scalar2=None, op0=Op.mult)
                    rel1.append(rl)
                areas = []
                for s in range(2):
                    ar = new(f"area{s}")
                    nc.vector.tensor_tensor(out=ar[:], in0=rel0[:], in1=rel1[s][:],
                                            op=Op.mult)
                    nc.vector.scalar_tensor_tensor(out=ar[:], in0=ar[:], scalar=-1.0,
                                                   in1=ar[:], op0=Op.mult, op1=Op.max)
                    nc.vector.tensor_scalar(out=ar[:], in0=ar[:],
                                            scalar1=float(np.float32(1e-9)),
                                            scalar2=None, op0=Op.add)
                    areas.append(ar)
                tot = new("tot")
                nc.vector.tensor_tensor(out=tot[:], in0=areas[0][:], in1=areas[1][:],
                                        op=Op.add)
                rtot = new("rtot")
                nc.vector.reciprocal(rtot[:], tot[:])

                idxs = []
                for s, ixf in ((0, ix0_f), (1, ix1_f)):
                    idf = cmt.tile([128, NB], F32, name="idf")
                    nc.vector.tensor_scalar(out=idf[:], in0=iy_f[:], scalar1=128.0,
                                            scalar2=None, op0=Op.mult)
                    nc.vector.tensor_tensor(out=idf[:], in0=idf[:], in1=ixf[:],
                                            op=Op.add)
                    idi = cm.tile([128, NB], I32, name=f"idx{s}")
                    nc.vector.tensor_copy(idi[:], idf[:])
                    idxs.append(idi)

                # g2 buffers: per (block b): [64 gathered ch | rel0 rel1 rc0 rc1
                #  ones | a]
                g2 = []
                for s in range(2):
                    g2s = cm.tile([128, NB, 70], F32, name=f"g2_{s}")
                    nc.vector.tensor_copy(g2s[:, :, 64], rel0[:])
                    nc.vector.tensor_copy(g2s[:, :, 65], rel1[s][:])
                    nc.vector.tensor_scalar(out=g2s[:, :, 66], in0=cellA_s[:, :, 0],
                                            scalar1=128.0, scalar2=None, op0=Op.mult)
                    nc.vector.tensor_scalar(out=g2s[:, :, 67], in0=cellA_s[:, :, 1],
                                            scalar1=128.0, scalar2=None, op0=Op.mult)
                    nc.vector.memset(g2s[:, :, 68], 1.0)
                    nc.vector.tensor_tensor(out=g2s[:, :, 69], in0=areas[s][:],
                                            in1=rtot[:], op=Op.mult)
                    g2.append(g2s)

                # ---------------- phase B: main query loop ----------------
                with tc.tile_pool(name="xb", bufs=3) as xbp, \
                     tc.tile_pool(name="hh", bufs=4) as hh, \
                     tc.tile_pool(name="ob", bufs=3) as ob, \
                     tc.tile_pool(name="pt", bufs=1, space="PSUM") as ppt, \
                     tc.tile_pool(name="pa", bufs=1, space="PSUM") as ppa, \
                     tc.tile_pool(name="phB", bufs=2, space="PSUM") as pphB, \
                     tc.tile_pool(name="poB", bufs=2, space="PSUM") as ppo:
                    for rnd in range(NB // 2):
                        xbuf = xbp.tile([70, 512], F32, name="xbuf")
                        for u in range(4):
                            b = rnd * 2 + u // 2
                            s = u % 2
                            nc.gpsimd.indirect_dma_start(
                                out=g2[s][:, b, 0:64],
                                out_offset=None,
                                in_=featfull[:, :],
                                in_offset=bass.IndirectOffsetOnAxis(
                                    ap=idxs[s][:, b:b + 1], axis=0),
                            )
                            tp = ppt.tile([70, 128], F32, name="tp")
                            nc.tensor.transpose(tp[:], g2[s][:, b, 0:70], ident[:])
                            nc.vector.tensor_copy(
                                xbuf[:, u * 128:(u + 1) * 128], tp[:])
                        abc = ppa.tile([128, 512], F32, name="abc")
                        nc.tensor.matmul(abc[:], lhsT=sel70[:], rhs=xbuf[:],
                                         start=True, stop=True)
                        abc_s = xbp.tile([128, 512], F32, name="abc_s",
                                         tag="abc_s")
                        nc.vector.tensor_copy(abc_s[:], abc[:])
                        for net, (w1n, w2n) in enumerate(
                                ((w1kk_s, w2kk_s), (w1vv_s, w2vv_s))):
                            hsn = []
                            for m in range(2):
                                ph = pphB.tile([128, 512], F32, name="phB")
                                nc.tensor.matmul(
                                    ph[:], lhsT=w1n[:, m * 128:(m + 1) * 128],
                                    rhs=xbuf[:], start=True, stop=True)
                                hm = hh.tile([128, 512], F32, name="hB")
                                nc.vector.scalar_tensor_tensor(
                                    out=hm[:], in0=ph[:], scalar=0.0, in1=abc_s[:],
                                    op0=Op.max, op1=Op.mult)
                                hsn.append(hm)
                            for bb in range(2):
                                po = ppo.tile([128, 576], F32, name="poB")
                                for (n0, n1) in ((0, 512), (512, 576)):
                                    first = True
                                    for m in range(2):
                                        for sc in range(2):
                                            col = (bb * 2 + sc) * 128
                                            nc.tensor.matmul(
                                                po[:, n0:n1],
                                                lhsT=hsn[m][:, col:col + 128],
                                                rhs=w2n[:, m, n0:n1],
                                                start=first,
                                                stop=(m == 1 and sc == 1))
                                            first = False
                                osb = ob.tile([128, 576], F32, name="osb")
                                nc.scalar.copy(osb[:], po[:])
                                qrow = (rnd * 2 + bb) * 128
                                nc.sync.dma_start(
                                    out_d[qrow:qrow + 128,
                                          net * 576:(net + 1) * 576],
                                    osb[:])

    nc.compile()
    return nc


# ---------------------------------------------------------------------------
# host-side input prep

def _prep_shared(inputs):
    f32 = np.float32
    coef_w = np.asarray(inputs["coef_w"], f32)
    coef_b = np.asarray(inputs["coef_b"], f32)
    freq_w = np.asarray(inputs["freq_w"], f32)
    freq_b = np.asarray(inputs["freq_b"], f32)
    phase_w = np.asarray(inputs["phase_w"], f32)
    phase_b = np.asarray(inputs["phase_b"], f32)
    qfw = freq_w[0::2] + freq_w[1::2] + phase_w        # [32, 64, 3, 3]
    qfb = freq_b[0::2] + freq_b[1::2] + phase_b        # [32]

    def taps(w):  # w [O, I, 3, 3] -> per-dy lhsT [I, 3(dx), O]
        return w.transpose(1, 3, 0, 2)                  # [I, kx, O, ky]

    cw = taps(coef_w)                                   # [64, 3, 64, 3]
    qw = taps(qfw)                                      # [64, 3, 32, 3]
    wa1 = np.concatenate([cw[..., 0], cw[..., 1]], axis=0).copy()   # [128,3,64]
    wb1 = cw[..., 2].copy()                                         # [64,3,64]
    qdup0 = np.concatenate([qw[..., 0], qw[..., 0]], axis=2)        # [64,3,64]
    qdup1 = np.concatenate([qw[..., 1], qw[..., 1]], axis=2)
    qdup2 = np.concatenate([qw[..., 2], qw[..., 2]], axis=2)
    wa2 = np.concatenate([qdup0, qdup1], axis=0).copy()             # [128,3,64]
    wb2 = qdup2.copy()                                              # [64,3,64]

    w1i = np.asarray(inputs["imnet_w1"], f32)           # [64, 256]
    b1i = np.asarray(inputs["imnet_b1"], f32).reshape(2, 128).T.copy()  # [128,2]
    w2i = np.asarray(inputs["imnet_w2"], f32).reshape(2, 128, 64)
    w2i = w2i.transpose(1, 0, 2).copy()                 # [128, 2, 64]
    b2i = np.asarray(inputs["imnet_b2"], f32).reshape(1, 64)

    def aug(w1, b1):
        # rows 0-67: w1; row 68: b1 (multiplied by the constant ones row of
        # xbuf); row 69: zeros (the area-weight row of xbuf contributes 0)
        return np.concatenate([np.asarray(w1, f32),
                               np.asarray(b1, f32)[None, :],
                               np.zeros((1, 256), f32)], axis=0)

    w1kk = aug(inputs["kk_w1"], inputs["kk_b1"])        # [70, 256]
    w1vv = aug(inputs["vv_w1"], inputs["vv_b1"])
    w2kk = np.asarray(inputs["kk_w2"], f32).reshape(2, 128, 576).transpose(1, 0, 2).copy()
    w2vv = np.asarray(inputs["vv_w2"], f32).reshape(2, 128, 576).transpose(1, 0, 2).copy()

    img = np.asarray(inputs["feat_i"], f32)[0]          # [64, 128, 128]
    pad = np.zeros((64, H + 2, W + 2), f32)
    pad[:, 1:H + 1, 1:W + 1] = img

    shared = dict(
        wa1=wa1, wb1=wb1, wa2=wa2, wb2=wb2,
        cbias=coef_b.reshape(64, 1),
        qbias=np.concatenate([qfb, qfb]).reshape(64, 1).astype(f32),
        w1i=w1i, b1i=b1i, w2i=w2i, b2i=b2i,
        w1kk=w1kk, w1vv=w1vv, w2kk=w2kk, w2vv=w2vv,
        sel70=np.where(np.arange(70)[:, None] == 69,
                       np.ones((70, 128), f32), np.zeros((70, 128), f32)),
    )
    return shared, pad


def _prep_core(inputs, pad, core):
    f32 = np.float32
    r0 = core * ROWS
    slab2 = np.empty((128, ROWS, W + 2), f32)
    slab2[0:64] = pad[:, r0:r0 + ROWS, :]
    slab2[64:128] = pad[:, r0 + 1:r0 + ROWS + 1, :]
    slab_c = pad[:, r0 + 2:r0 + ROWS + 2, :].copy()

    coord = np.asarray(inputs["coord"], f32)[0]         # [65536, 2]
    cell = np.asarray(inputs["cell"], f32)[0]
    cs = coord[core * QS:(core + 1) * QS].reshape(NB, 128, 2)
    ce = cell[core * QS:(core + 1) * QS].reshape(NB, 128, 2)
    coordA = cs.transpose(1, 0, 2).copy()               # [128, NB, 2]
    cellA = ce.transpose(1, 0, 2).copy()
    return dict(slab2=slab2, slab_c=slab_c, coordA=coordA, cellA=cellA)


_CACHED_NC = [None]


def kernel(**inputs):
    if _CACHED_NC[0] is None:
        _CACHED_NC[0] = _build_program()
    nc = _CACHED_NC[0]

    shared, pad = _prep_shared(inputs)
    in_maps = []
    for core in range(NCORES):
        m = dict(shared)
        m.update(_prep_core(inputs, pad, core))
        in_maps.append(m)

    res = run_bass_kernel_spmd(nc, in_maps, core_ids=list(range(NCORES)))
    out = np.concatenate([res.results[c]["out"] for c in range(NCORES)], axis=0)
    out[:, 0:576] += np.asarray(inputs["kk_b2"], np.float32)[None, :]
    out[:, 576:1152] += np.asarray(inputs["vv_b2"], np.float32)[None, :]
    return out.reshape(1, Q, 1152)


# revision 16
# speedup vs baseline: 2.3024x; 2.3024x over previous
"""CiaoSR query-decode kernel for 8 Trainium2 NeuronCores.

Strategy (data-parallel over query points Q):
  - gen_feat (convs + imnet MLP over the 128x128 feature map) is sharded by
    image rows (16 rows/core); the per-pixel feature map [16384, 64] is
    AllGathered so every core holds the full map.
  - The two grid samples per query share iy (vx=-1 only), so each query
    needs rows idx0 = iy*W+ix0 and idx1 = iy*W+ix1 of the pixel-major map.
    Rows are fetched with one [128-partition, 1-offset] indirect DMA per
    128-query block per sample offset.
  - kk/vv MLPs (68->256->576) run as PE matmuls; per-query area weights are
    folded into the hidden activations so the two sample offsets accumulate
    in one PSUM tile; kk_b2/vv_b2 are added on the host (their reference
    coefficient a0+a1 == 1 up to 1 ulp).
"""

import numpy as np

import bass_rust
import concourse.bass as bass
import concourse.mybir as mybir
import concourse.tile as tile
from concourse import bacc
from concourse.bass_utils import run_bass_kernel_spmd
from concourse.masks import make_identity
from concourse.vector_clock import ScopedClock

F32 = mybir.dt.float32
I32 = mybir.dt.int32
F32R = mybir.dt.float32r
Op = mybir.AluOpType
Act = mybir.ActivationFunctionType

H = W = 128
C = 64
Q = 65536
NCORES = 8
QS = Q // NCORES          # 8192 queries per core
NB = QS // 128            # 64 blocks of 128 queries
ROWS = H // NCORES        # 16 image rows per core
PIX = ROWS * W            # 2048 pixels per core
RX = 1.0 / H
EPS = 1e-6
PI = float(np.pi)

# ---------------------------------------------------------------------------
# walrus wait-capacity workaround: hoist excess per-instruction sem waits
# onto standalone event-semaphore nops (this walrus accepts 1 wait per
# instruction, 2 on EventSemaphore).
_wait_ctr = [0]


def _legalize_waits(nc):
    for fn in nc.m.functions:
        for bb in fn.blocks:
            insts = bb.instructions
            out = []
            changed = False
            for inst in insts:
                si = inst.sync_info
                waits = list(si.on_wait) if si is not None else []
                cap = 2 if isinstance(inst, mybir.InstEventSemaphore) else 1
                if len(waits) > cap:
                    changed = True
                    extra, keep = waits[:-cap], waits[-cap:]
                    for i in range(0, len(extra), 2):
                        _wait_ctr[0] += 1
                        out.append(
                            mybir.InstEventSemaphore(
                                name=f"I-waitfix-{_wait_ctr[0]}",
                                engine=inst.engine,
                                ins=[],
                                outs=[],
                                sync_info=bass_rust.SyncInfo(
                                    on_wait=extra[i : i + 2], on_update=[]
                                ),
                            )
                        )
                    si.on_wait = keep
                    inst.sync_info = si
                out.append(inst)
            if changed:
                bb.instructions = out


_patched = [False]


def _patch_tile():
    if _patched[0]:
        return
    _patched[0] = True
    orig_exit = tile.TileContext.__exit__

    def patched_exit(self, exc_type, exc, tb):
        r = orig_exit(self, exc_type, exc, tb)
        if exc_type is None:
            _legalize_waits(self.nc)
        return r

    tile.TileContext.__exit__ = patched_exit


# ---------------------------------------------------------------------------
# program construction


def _build_program():
    _patch_tile()
    nc = bacc.Bacc("TRN2", target_bir_lowering=False, debug=False,
                   num_devices=NCORES)

    def ein(name, shape):
        return nc.dram_tensor(name, shape, F32, kind="ExternalInput")

    slab2 = ein("slab2", [128, ROWS, W + 2])
    slab_c = ein("slab_c", [64, ROWS, W + 2])
    wa1 = ein("wa1", [128, 3, 64])
    wb1 = ein("wb1", [64, 3, 64])
    wa2 = ein("wa2", [128, 3, 64])
    wb2 = ein("wb2", [64, 3, 64])
    cbias = ein("cbias", [64, 1])
    qbias = ein("qbias", [64, 1])
    w1i = ein("w1i", [64, 256])
    b1i = ein("b1i", [128, 2])
    w2i = ein("w2i", [128, 2, 64])
    b2i = ein("b2i", [1, 64])
    w1kk = nc.dram_tensor("w1kk", [70, 256], F32R, kind="ExternalInput")
    w1vv = nc.dram_tensor("w1vv", [70, 256], F32R, kind="ExternalInput")
    w2kk = nc.dram_tensor("w2kk", [128, 2, 576], F32R, kind="ExternalInput")
    w2vv = nc.dram_tensor("w2vv", [128, 2, 576], F32R, kind="ExternalInput")
    coordA = ein("coordA", [128, NB, 2])
    cellA = ein("cellA", [128, NB, 2])
    # selector: sel70.T @ xbuf broadcasts xbuf row 69 (the area weights)
    # to all 128 output partitions
    sel70_d = nc.dram_tensor("sel70", [70, 128], F32R, kind="ExternalInput")
    out_d = nc.dram_tensor("out", [QS, 1152], F32, kind="ExternalOutput")

    with tile.TileContext(nc) as tc:
        with tc.tile_pool(name="const", bufs=1) as cp, \
             tc.tile_pool(name="dram", bufs=1, space="DRAM") as dp:
            featsh = dp.tile([PIX, C], F32)
            featfull = dp.tile([H * W, C], F32, addr_space="Shared")

            ident = cp.tile([128, 128], F32)
            make_identity(nc, ident[:])
            ones1 = cp.tile([1, 128], F32)
            nc.vector.memset(ones1[:], 1.0)


            def load(src, shape, dt=F32):
                t = cp.tile(shape, dt, name=src.name + "_sb")
                nc.sync.dma_start(t[:], src[(slice(None),) * len(shape)])
                return t

            slab2_s = load(slab2, [128, ROWS, W + 2])
            slabc_s = load(slab_c, [64, ROWS, W + 2])
            wa1_s = load(wa1, [128, 3, 64])
            wb1_s = load(wb1, [64, 3, 64])
            wa2_s = load(wa2, [128, 3, 64])
            wb2_s = load(wb2, [64, 3, 64])
            cbias_s = load(cbias, [64, 1])
            qbias_s = load(qbias, [64, 1])
            w1i_s = load(w1i, [64, 256])
            b1i_s = load(b1i, [128, 2])
            w2i_s = load(w2i, [128, 2, 64])
            b2i_s = load(b2i, [1, 64])
            w1kk_s = load(w1kk, [70, 256], F32R)
            w1vv_s = load(w1vv, [70, 256], F32R)
            w2kk_s = load(w2kk, [128, 2, 576], F32R)
            w2vv_s = load(w2vv, [128, 2, 576], F32R)
            coordA_s = load(coordA, [128, NB, 2])
            cellA_s = load(cellA, [128, NB, 2])
            sel70 = load(sel70_d, [70, 128], F32R)

            # ---------------- phase A: gen_feat ----------------
            with tc.tile_pool(name="ga", bufs=3) as ga, \
                 tc.tile_pool(name="gh", bufs=4) as gh, \
                 tc.tile_pool(name="pc", bufs=2, space="PSUM") as ppc, \
                 tc.tile_pool(name="pq", bufs=2, space="PSUM") as ppq, \
                 tc.tile_pool(name="phA", bufs=2, space="PSUM") as pph, \
                 tc.tile_pool(name="pfA", bufs=2, space="PSUM") as ppf:
                for yg in range(ROWS // 4):
                    y0 = yg * 4
                    pc = ppc.tile([64, 4, 128], F32, name="pc")
                    pq = ppq.tile([64, 4, 128], F32, name="pq")
                    for po, wa, wb in ((pc, wa1_s, wb1_s), (pq, wa2_s, wb2_s)):
                        for dx in range(3):
                            nc.tensor.matmul(
                                po[:], lhsT=wa[:, dx, :],
                                rhs=slab2_s[:, y0:y0 + 4, dx:dx + 128],
                                start=(dx == 0), stop=False)
                        for dx in range(3):
                            nc.tensor.matmul(
                                po[:], lhsT=wb[:, dx, :],
                                rhs=slabc_s[:, y0:y0 + 4, dx:dx + 128],
                                start=False, stop=(dx == 2))
                    # trig: rows 0-31 cos(pi*qf) (via sin(pi*(qf+0.5-2k))),
                    # rows 32-63 sin(pi*qf)
                    u = ga.tile([64, 4, 128], F32, name="u")
                    # u = pq + qbias
                    nc.vector.tensor_scalar(
                        out=u[:], in0=pq[:], scalar1=qbias_s[:, 0:1],
                        scalar2=None, op0=Op.add)
                    # cos half: add 0.5
                    nc.vector.tensor_scalar(
                        out=u[0:32], in0=u[0:32], scalar1=0.5, scalar2=None,
                        op0=Op.add)
                    uh = ga.tile([64, 4, 128], F32, name="uh")
                    nc.vector.tensor_scalar(
                        out=uh[:], in0=u[:], scalar1=0.5, scalar2=None,
                        op0=Op.mult)
                    ki = ga.tile([64, 4, 128], I32, name="ki")
                    nc.vector.tensor_copy(ki[:], uh[:])
                    kf = ga.tile([64, 4, 128], F32, name="kf")
                    nc.vector.tensor_copy(kf[:], ki[:])
                    r = ga.tile([64, 4, 128], F32, name="r")
                    nc.vector.scalar_tensor_tensor(
                        out=r[:], in0=kf[:], scalar=-2.0, in1=u[:],
                        op0=Op.mult, op1=Op.add)
                    cossin = ga.tile([64, 4, 128], F32, name="cossin")
                    nc.scalar.activation(cossin[:], r[:], Act.Sin, scale=PI)
                    x2 = ga.tile([64, 4, 128], F32, name="x2")
                    nc.vector.scalar_tensor_tensor(
                        out=x2[:], in0=pc[:], scalar=cbias_s[:, 0:1],
                        in1=cossin[:], op0=Op.add, op1=Op.mult)
                    hs = []
                    for m in range(2):
                        ph = pph.tile([128, 512], F32, name="phA")
                        nc.tensor.matmul(
                            ph[:], lhsT=w1i_s[:, m * 128:(m + 1) * 128],
                            rhs=x2[:].rearrange("c a b -> c (a b)"),
                            start=True, stop=True)
                        hm = gh.tile([128, 512], F32, name="hA")
                        nc.scalar.activation(hm[:], ph[:], Act.Relu,
                                             bias=b1i_s[:, m:m + 1])
                        hs.append(hm)
                    for sub in range(4):
                        pf = ppf.tile([128, 64], F32, name="pfA")
                        nc.tensor.matmul(
                            pf[:], lhsT=hs[0][:, sub * 128:(sub + 1) * 128],
                            rhs=w2i_s[:, 0, :], start=True, stop=False)
                        nc.tensor.matmul(
                            pf[:], lhsT=hs[1][:, sub * 128:(sub + 1) * 128],
                            rhs=w2i_s[:, 1, :], start=False, stop=False)
                        nc.tensor.matmul(
                            pf[:], lhsT=ones1[:], rhs=b2i_s[:],
                            start=False, stop=True)
                        ft = ga.tile([128, 64], F32, name="ft")
                        nc.scalar.copy(ft[:], pf[:])
                        r0 = yg * 512 + sub * 128
                        nc.sync.dma_start(featsh[r0:r0 + 128, :], ft[:])

            nc.gpsimd.collective_compute(
                "AllGather",
                Op.bypass,
                replica_groups=[list(range(NCORES))],
                ins=[featsh[:, :]],
                outs=[featfull[:, :]],
            )

            # ---------------- phase B: coord math ----------------
            with tc.tile_pool(name="cm", bufs=1) as cm, \
                 tc.tile_pool(name="cmt", bufs=2) as cmt:
                c0 = coordA_s[:, :, 0]
                c1 = coordA_s[:, :, 1]

                def new(name):
                    return cm.tile([128, NB], F32, name=name)

                def pixel_center_chain(csrc, off_const, name):
                    """returns (i_clipped_f32, qc) replicating jnp op order"""
                    t = new(name + "_t")
                    nc.vector.tensor_scalar(out=t[:], in0=csrc, scalar1=float(off_const),
                                            scalar2=None, op0=Op.add)
                    nc.vector.tensor_scalar(out=t[:], in0=t[:], scalar1=float(np.float32(EPS)),
                                            scalar2=None, op0=Op.add)
                    nc.vector.tensor_scalar(out=t[:], in0=t[:],
                                            scalar1=float(np.float32(-1 + EPS)),
                                            scalar2=float(np.float32(1 - EPS)),
                                            op0=Op.max, op1=Op.min)
                    nc.vector.tensor_scalar(out=t[:], in0=t[:], scalar1=1.0,
                                            scalar2=None, op0=Op.add)
                    nc.vector.tensor_scalar(out=t[:], in0=t[:], scalar1=128.0,
                                            scalar2=None, op0=Op.mult)
                    nc.vector.tensor_scalar(out=t[:], in0=t[:], scalar1=-1.0,
                                            scalar2=None, op0=Op.add)
                    nc.vector.tensor_scalar(out=t[:], in0=t[:], scalar1=0.5,
                                            scalar2=None, op0=Op.mult)
                    ti = cm.tile([128, NB], I32, name=name + "_i")
                    nc.vector.tensor_copy(ti[:], t[:])
                    tf = new(name + "_f")
                    nc.vector.tensor_copy(tf[:], ti[:])
                    nc.vector.tensor_scalar(out=tf[:], in0=tf[:], scalar1=0.0,
                                            scalar2=127.0, op0=Op.max, op1=Op.min)
                    qc = new(name + "_qc")
                    nc.vector.tensor_scalar(out=qc[:], in0=tf[:], scalar1=2.0,
                                            scalar2=1.0, op0=Op.mult, op1=Op.add)
                    nc.vector.tensor_scalar(out=qc[:], in0=qc[:],
                                            scalar1=float(np.float32(1.0 / 128.0)),
                                            scalar2=-1.0, op0=Op.mult, op1=Op.add)
                    return tf, qc

                iy_f, qcy = pixel_center_chain(c0, np.float32(-RX), "iy")
                ix0_f, qcx0 = pixel_center_chain(c1, np.float32(-RX), "ix0")
                ix1_f, qcx1 = pixel_center_chain(c1, np.float32(RX), "ix1")

                rel0 = new("rel0")
                nc.vector.tensor_tensor(out=rel0[:], in0=c0, in1=qcy[:],
                                        op=Op.subtract)
                nc.vector.tensor_scalar(out=rel0[:], in0=rel0[:], scalar1=128.0,
                                        scalar2=None, op0=Op.mult)
                rel1 = []
                for s, qcx in ((0, qcx0), (1, qcx1)):
                    rl = new(f"rel1_{s}")
                    nc.vector.tensor_tensor(out=rl[:], in0=c1, in1=qcx[:],
                                            op=Op.subtract)
                    nc.vector.tensor_scalar(out=rl[:], in0=rl[:], scalar1=128.0,
                                            scalar2=None, op0=Op.mult)
                    rel1.append(rl)
                areas = []
                for s in range(2):
                    ar = new(f"area{s}")
                    nc.vector.tensor_tensor(out=ar[:], in0=rel0[:], in1=rel1[s][:],
                                            op=Op.mult)
                    nc.vector.scalar_tensor_tensor(out=ar[:], in0=ar[:], scalar=-1.0,
                                                   in1=ar[:], op0=Op.mult, op1=Op.max)
                    nc.vector.tensor_scalar(out=ar[:], in0=ar[:],
                                            scalar1=float(np.float32(1e-9)),
                                            scalar2=None, op0=Op.add)
                    areas.append(ar)
                tot = new("tot")
                nc.vector.tensor_tensor(out=tot[:], in0=areas[0][:], in1=areas[1][:],
                                        op=Op.add)
                rtot = new("rtot")
                nc.vector.reciprocal(rtot[:], tot[:])

                idxs = []
                for s, ixf in ((0, ix0_f), (1, ix1_f)):
                    idf = cmt.tile([128, NB], F32, name="idf")
                    nc.vector.tensor_scalar(out=idf[:], in0=iy_f[:], scalar1=128.0,
                                            scalar2=None, op0=Op.mult)
                    nc.vector.tensor_tensor(out=idf[:], in0=idf[:], in1=ixf[:],
                                            op=Op.add)
                    idi = cm.tile([128, NB], I32, name=f"idx{s}")
                    nc.vector.tensor_copy(idi[:], idf[:])
                    idxs.append(idi)

                # g2 buffers: per (block b): [64 gathered ch | rel0 rel1 rc0 rc1
                #  ones | a]
                g2 = []
                for s in range(2):
                    g2s = cm.tile([128, NB, 70], F32, name=f"g2_{s}")
                    nc.vector.tensor_copy(g2s[:, :, 64], rel0[:])
                    nc.vector.tensor_copy(g2s[:, :, 65], rel1[s][:])
                    nc.vector.tensor_scalar(out=g2s[:, :, 66], in0=cellA_s[:, :, 0],
                                            scalar1=128.0, scalar2=None, op0=Op.mult)
                    nc.vector.tensor_scalar(out=g2s[:, :, 67], in0=cellA_s[:, :, 1],
                                            scalar1=128.0, scalar2=None, op0=Op.mult)
                    nc.vector.memset(g2s[:, :, 68], 1.0)
                    nc.vector.tensor_tensor(out=g2s[:, :, 69], in0=areas[s][:],
                                            in1=rtot[:], op=Op.mult)
                    g2.append(g2s)

                # ---------------- phase B: main query loop ----------------
                with tc.tile_pool(name="xb", bufs=3) as xbp, \
                     tc.tile_pool(name="hh", bufs=4) as hh, \
                     tc.tile_pool(name="ob", bufs=3) as ob, \
                     tc.tile_pool(name="pt", bufs=1, space="PSUM") as ppt, \
                     tc.tile_pool(name="pa", bufs=1, space="PSUM") as ppa, \
                     tc.tile_pool(name="phB", bufs=2, space="PSUM") as pphB, \
                     tc.tile_pool(name="poB", bufs=2, space="PSUM") as ppo:
                    for rnd in range(NB // 2):
                        xbuf = xbp.tile([70, 512], F32R, name="xbuf")
                        for u in range(4):
                            b = rnd * 2 + u // 2
                            s = u % 2
                            nc.gpsimd.indirect_dma_start(
                                out=g2[s][:, b, 0:64],
                                out_offset=None,
                                in_=featfull[:, :],
                                in_offset=bass.IndirectOffsetOnAxis(
                                    ap=idxs[s][:, b:b + 1], axis=0),
                            )
                            tp = ppt.tile([70, 128], F32, name="tp")
                            nc.tensor.transpose(tp[:], g2[s][:, b, 0:70], ident[:])
                            nc.vector.tensor_copy(
                                xbuf[:, u * 128:(u + 1) * 128], tp[:])
                        abc = ppa.tile([128, 512], F32, name="abc")
                        nc.tensor.matmul(abc[:], lhsT=sel70[:],
                                         rhs=xbuf[:],
                                         start=True, stop=True)
                        abc_s = xbp.tile([128, 512], F32, name="abc_s",
                                         tag="abc_s")
                        nc.vector.tensor_copy(abc_s[:], abc[:])
                        for net, (w1n, w2n) in enumerate(
                                ((w1kk_s, w2kk_s), (w1vv_s, w2vv_s))):
                            hsn = []
                            for m in range(2):
                                ph = pphB.tile([128, 512], F32, name="phB")
                                nc.tensor.matmul(
                                    ph[:],
                                    lhsT=w1n[:, m * 128:(m + 1) * 128],
                                    rhs=xbuf[:],
                                    start=True, stop=True)
                                hm = hh.tile([128, 512], F32R, name="hB")
                                nc.vector.scalar_tensor_tensor(
                                    out=hm[:], in0=ph[:], scalar=0.0, in1=abc_s[:],
                                    op0=Op.max, op1=Op.mult)
                                hsn.append(hm)
                            for bb in range(2):
                                po = ppo.tile([128, 576], F32, name="poB")
                                for (n0, n1) in ((0, 512), (512, 576)):
                                    first = True
                                    for m in range(2):
                                        for sc in range(2):
                                            col = (bb * 2 + sc) * 128
                                            nc.tensor.matmul(
                                                po[:, n0:n1],
                                                lhsT=hsn[m][:, col:col + 128],
                                                rhs=w2n[:, m, n0:n1],
                                                start=first,
                                                stop=(m == 1 and sc == 1))
                                            first = False
                                osb = ob.tile([128, 576], F32, name="osb")
                                nc.scalar.copy(osb[:], po[:])
                                qrow = (rnd * 2 + bb) * 128
                                nc.sync.dma_start(
                                    out_d[qrow:qrow + 128,
                                          net * 576:(net + 1) * 576],
                                    osb[:])

    nc.compile()
    return nc


# ---------------------------------------------------------------------------
# host-side input prep

def _prep_shared(inputs):
    f32 = np.float32
    coef_w = np.asarray(inputs["coef_w"], f32)
    coef_b = np.asarray(inputs["coef_b"], f32)
    freq_w = np.asarray(inputs["freq_w"], f32)
    freq_b = np.asarray(inputs["freq_b"], f32)
    phase_w = np.asarray(inputs["phase_w"], f32)
    phase_b = np.asarray(inputs["phase_b"], f32)
    qfw = freq_w[0::2] + freq_w[1::2] + phase_w        # [32, 64, 3, 3]
    qfb = freq_b[0::2] + freq_b[1::2] + phase_b        # [32]

    def taps(w):  # w [O, I, 3, 3] -> per-dy lhsT [I, 3(dx), O]
        return w.transpose(1, 3, 0, 2)                  # [I, kx, O, ky]

    cw = taps(coef_w)                                   # [64, 3, 64, 3]
    qw = taps(qfw)                                      # [64, 3, 32, 3]
    wa1 = np.concatenate([cw[..., 0], cw[..., 1]], axis=0).copy()   # [128,3,64]
    wb1 = cw[..., 2].copy()                                         # [64,3,64]
    qdup0 = np.concatenate([qw[..., 0], qw[..., 0]], axis=2)        # [64,3,64]
    qdup1 = np.concatenate([qw[..., 1], qw[..., 1]], axis=2)
    qdup2 = np.concatenate([qw[..., 2], qw[..., 2]], axis=2)
    wa2 = np.concatenate([qdup0, qdup1], axis=0).copy()             # [128,3,64]
    wb2 = qdup2.copy()                                              # [64,3,64]

    w1i = np.asarray(inputs["imnet_w1"], f32)           # [64, 256]
    b1i = np.asarray(inputs["imnet_b1"], f32).reshape(2, 128).T.copy()  # [128,2]
    w2i = np.asarray(inputs["imnet_w2"], f32).reshape(2, 128, 64)
    w2i = w2i.transpose(1, 0, 2).copy()                 # [128, 2, 64]
    b2i = np.asarray(inputs["imnet_b2"], f32).reshape(1, 64)

    def aug(w1, b1):
        # rows 0-67: w1; row 68: b1 (multiplied by the constant ones row of
        # xbuf); row 69: zeros (the area-weight row of xbuf contributes 0)
        return np.concatenate([np.asarray(w1, f32),
                               np.asarray(b1, f32)[None, :],
                               np.zeros((1, 256), f32)], axis=0)

    w1kk = aug(inputs["kk_w1"], inputs["kk_b1"])        # [70, 256]
    w1vv = aug(inputs["vv_w1"], inputs["vv_b1"])
    w2kk = np.asarray(inputs["kk_w2"], f32).reshape(2, 128, 576).transpose(1, 0, 2).copy()
    w2vv = np.asarray(inputs["vv_w2"], f32).reshape(2, 128, 576).transpose(1, 0, 2).copy()

    img = np.asarray(inputs["feat_i"], f32)[0]          # [64, 128, 128]
    pad = np.zeros((64, H + 2, W + 2), f32)
    pad[:, 1:H + 1, 1:W + 1] = img

    shared = dict(
        wa1=wa1, wb1=wb1, wa2=wa2, wb2=wb2,
        cbias=coef_b.reshape(64, 1),
        qbias=np.concatenate([qfb, qfb]).reshape(64, 1).astype(f32),
        w1i=w1i, b1i=b1i, w2i=w2i, b2i=b2i,
        w1kk=w1kk, w1vv=w1vv, w2kk=w2kk, w2vv=w2vv,
        sel70=np.where(np.arange(70)[:, None] == 69,
                       np.ones((70, 128), f32), np.zeros((70, 128), f32)),
    )
    return shared, pad


def _prep_core(inputs, pad, core):
    f32 = np.float32
    r0 = core * ROWS
    slab2 = np.empty((128, ROWS, W + 2), f32)
    slab2[0:64] = pad[:, r0:r0 + ROWS, :]
    slab2[64:128] = pad[:, r0 + 1:r0 + ROWS + 1, :]
    slab_c = pad[:, r0 + 2:r0 + ROWS + 2, :].copy()

    coord = np.asarray(inputs["coord"], f32)[0]         # [65536, 2]
    cell = np.asarray(inputs["cell"], f32)[0]
    cs = coord[core * QS:(core + 1) * QS].reshape(NB, 128, 2)
    ce = cell[core * QS:(core + 1) * QS].reshape(NB, 128, 2)
    coordA = cs.transpose(1, 0, 2).copy()               # [128, NB, 2]
    cellA = ce.transpose(1, 0, 2).copy()
    return dict(slab2=slab2, slab_c=slab_c, coordA=coordA, cellA=cellA)


_CACHED_NC = [None]


def kernel(**inputs):
    if _CACHED_NC[0] is None:
        _CACHED_NC[0] = _build_program()
    nc = _CACHED_NC[0]

    shared, pad = _prep_shared(inputs)
    in_maps = []
    for core in range(NCORES):
        m = dict(shared)
        m.update(_prep_core(inputs, pad, core))
        in_maps.append(m)

    res = run_bass_kernel_spmd(nc, in_maps, core_ids=list(range(NCORES)))
    out = np.concatenate([res.results[c]["out"] for c in range(NCORES)], axis=0)
    out[:, 0:576] += np.asarray(inputs["kk_b2"], np.float32)[None, :]
    out[:, 576:1152] += np.asarray(inputs["vv_b2"], np.float32)[None, :]
    return out.reshape(1, Q, 1152)
